# revision 11
# baseline (speedup 1.0000x reference)
"""Multi-head attention (B=2, S=2048, H=2048, NH=16, HD=128) on 8 trn2 cores.

Sharding: core i -> (batch b = i // 4, head-group g = i % 4, 4 heads each).
Each core computes q/k/v projections for its 4 heads, causal-masked
attention, and a partial output projection against its 512-row slice of
Wo.  The host sums the 4 partial outputs per batch (partials stored in
bf16 to halve the store traffic).

Layout strategy (everything K-major so no on-chip transposes are needed):
  - host ships x^T (per batch) in bf16; projections compute q^T/k^T
    [d, t] via lhsT=W, rhs=x^T, and v [T, d] via lhsT=x^T, rhs=Wv.
  - scores^T [T, t] = (k^T).T @ q^T; exp on ACT (no max subtraction --
    scores are O(6) here, exp is safe in fp32).
  - diagonal (causal-staircase) blocks are SUFFIX-TRIMMED: only query
    columns [delta:512] are computed by the score matmul / exp / AV
    matmul (delta = Tb*128 - tau*512).  The 128-wide boundary strip is
    masked by multiplying with a host-precomputed staircase tile
    (stair[p, j] = j >= p, identical for every diagonal block); the
    dead prefix of the e tile is zeroed with a gpsimd memset so the
    esum tree stays full-width.
  - softmax denominators: e tiles accumulate on DVE into a bf16 esum
    tree, reduced across partitions with one ones-matmul per (head,
    block); the reciprocal row r[1,512] is broadcast across partitions
    with a second K=1 ones-matmul on the PE (out[p, t] = 1 * r[t]) --
    no DRAM bounce, no gpsimd ucode.
  - final: out[t, m] = (o^T).T @ Wo_rows, accumulated over the 4 heads,
    stored bf16 via the gpsimd DMA queue.

DMA queues: sync carries the two latency-critical first chunks (wq k0,
xT k0) then the xT stream; vector carries wk; scalar carries the stair
tile, wv, then wo; gpsimd carries the rest of wq plus output stores.
First chunks are 1 k-tile (128 KB) so the first matmul can start as
soon as the framework preamble ends.

Emission is software-pipelined: tau=0 projections round-robin q/k/v so
all four DMA queues drain in parallel; in query-block tau's slot we
emit its attention heads round-robin with the projections of tau+1 and
output-projection rows, pacing fillers proportionally and holding a
reserve back so the PE has work while the last head's normalization
chain completes.
"""

import math

import numpy as np
import ml_dtypes

B, S, H, NH, HD = 2, 2048, 2048, 16, 128
N_CORES = 8
GROUPS = 4                # head-groups (cores per batch)
HPC = NH // GROUPS        # heads per core = 4
DPC = HPC * HD            # head dims per core = 512
TBLK = 512                # query-block width (matmul moving dim)
KBLK = 128                # key-block width (matmul contraction dim)
NT = S // TBLK            # 4 query blocks
NK = S // KBLK            # 16 key blocks
HKT = H // 128            # 16 contraction tiles over hidden dim

_BF16 = ml_dtypes.bfloat16

_kernel_cache = {}


MODE_FULL, MODE_AFFINE, MODE_LOADMASK = 0, 1, 2


def _runs(blocks):
    """Group the load-mask blocks of one query block into contiguous Tb
    runs so each run loads with a single DMA."""
    runs = []
    for Tb, mode in blocks:
        if mode != MODE_LOADMASK:
            continue
        if runs and runs[-1][-1] == Tb - 1 and len(runs[-1]) < 4:
            runs[-1].append(Tb)
        else:
            runs.append([Tb])
    return runs


def _build(pattern):
    """Compile the SPMD program for a given mask block pattern.

    pattern: tuple over query-block tau of tuples of (Tb, mode) pairs,
    ascending in Tb, listing key blocks that have any visible entry.
    """
    import concourse.bass as bass  # noqa: F401
    import concourse.tile as tile
    from concourse import bacc, mybir

    fp32 = mybir.dt.float32
    bf16 = mybir.dt.bfloat16
    Exp = mybir.ActivationFunctionType.Exp
    inv_sqrt_hd = 1.0 / math.sqrt(HD)

    all_runs = [_runs(blocks) for blocks in pattern]
    max_run_len = max((len(r) for runs in all_runs for r in runs), default=1)
    max_runs = max((len(runs) for runs in all_runs), default=1)

    nc = bacc.Bacc("TRN2", target_bir_lowering=False, debug=False,
                   num_devices=N_CORES)
    xT = nc.dram_tensor("xT", [H, S], bf16, kind="ExternalInput")
    wq = nc.dram_tensor("wq", [H, DPC], bf16, kind="ExternalInput")
    wk = nc.dram_tensor("wk", [H, DPC], bf16, kind="ExternalInput")
    wv = nc.dram_tensor("wv", [H, DPC], bf16, kind="ExternalInput")
    wo = nc.dram_tensor("wo", [DPC, H], bf16, kind="ExternalInput")
    maskT = nc.dram_tensor("maskT", [S, S], bf16, kind="ExternalInput")
    stair = nc.dram_tensor("stair", [KBLK, KBLK], bf16, kind="ExternalInput")
    out = nc.dram_tensor("out", [S, H], bf16, kind="ExternalOutput")

    with tile.TileContext(nc) as tc:
        with (
            tc.tile_pool(name="persist", bufs=1) as persist,
            tc.tile_pool(name="xt", bufs=10) as xt_pool,
            tc.tile_pool(name="masks", bufs=max(2 * max_runs, 2)) as mask_pool,
            tc.tile_pool(name="e", bufs=9) as e_pool,
            tc.tile_pool(name="outsb", bufs=4) as out_pool,
            tc.tile_pool(name="esum", bufs=7) as esum_pool,
            tc.tile_pool(name="rp", bufs=2) as r_pool,
            tc.tile_pool(name="rbp", bufs=2) as rb_pool,
            tc.tile_pool(name="Rp", bufs=2) as R_pool,
            tc.tile_pool(name="ps_work", bufs=3, space="PSUM") as ps_work,
            tc.tile_pool(name="ps_score", bufs=3, space="PSUM") as ps_score,
            tc.tile_pool(name="ps_acc", bufs=2, space="PSUM") as ps_acc,
        ):
            # --- persistent SBUF tensors -------------------------------
            # stair tile first on the scalar queue (32 KB, needed by the
            # first diagonal attention block ~20us in)
            stair_sb = persist.tile([KBLK, KBLK], bf16, tag="stair")
            nc.scalar.dma_start(stair_sb[:], stair.ap()[:, :])

            # Weight/xT chunk plans: variable k-tile chunk sizes so the
            # first matmul's inputs are tiny and first-in-queue, spread
            # across the three DMA-capable queues (sync/SP, scalar/ACT,
            # gpsimd) so each projection phase is gated by a different
            # queue in arrival order: wq (gpsimd, + k0 on sync), wk
            # (scalar, behind the upper half of xT0), wv (gpsimd,
            # behind wq).
            w_sbs = {}      # name -> list of (tile, k0, nk)
            W_PLANS = {
                "wq": ([1, 1, 2, 4, 4, 4],
                       [nc.sync, nc.gpsimd, nc.gpsimd, nc.gpsimd,
                        nc.gpsimd, nc.gpsimd]),
                "wk": ([1, 1, 2, 4, 4, 4], [nc.scalar] * 6),
                "wv": ([2, 2, 4, 4, 4], [nc.gpsimd] * 5),
            }

            def load_w(name, dram):
                sizes, engines = W_PLANS[name]
                chunks = []
                k0 = 0
                for ci, s in enumerate(sizes):
                    t = persist.tile([128, s, DPC], bf16, tag=f"{name}{ci}")
                    engines[ci].dma_start(
                        t[:],
                        dram.ap()[k0 * 128:(k0 + s) * 128, :]
                        .rearrange("(k p) d -> p k d", p=128))
                    chunks.append((t, k0, s))
                    k0 += s
                assert k0 == HKT
                w_sbs[name] = chunks

            def w_chunk(name, hk):
                for t, k0, s in w_sbs[name]:
                    if k0 <= hk < k0 + s:
                        return t[:, hk - k0, :]
                raise KeyError(hk)

            # xT chunk loads.  tau=0 splits across sync (k0-7, small
            # leading chunks) and scalar (k8-15) so the q-phase isn't
            # gated by a single queue; later taus ride sync whole.
            XT0_PLAN = ([1, 1, 2, 4, 4, 4],
                        [nc.sync, nc.sync, nc.sync, nc.sync,
                         nc.scalar, nc.scalar])
            XT_SIZES = [4, 4, 4, 4]
            xts = {}        # tau -> list of (tile, k0, s)

            def emit_xt_load(tau):
                if tau in xts:
                    return
                if tau == 0:
                    sizes, engines = XT0_PLAN
                else:
                    sizes, engines = XT_SIZES, [nc.sync] * len(XT_SIZES)
                tsl = slice(tau * TBLK, (tau + 1) * TBLK)
                chunks = []
                k0 = 0
                for ci, s in enumerate(sizes):
                    t = xt_pool.tile([128, s, TBLK], bf16, tag="xt")
                    engines[ci].dma_start(
                        t[:],
                        xT.ap()[k0 * 128:(k0 + s) * 128, tsl]
                        .rearrange("(k p) t -> p k t", p=128))
                    chunks.append((t, k0, s))
                    k0 += s
                xts[tau] = chunks

            def xt_chunk(tau, hk):
                for t, k0, s in xts[tau]:
                    if k0 <= hk < k0 + s:
                        return t[:, hk - k0, :]
                raise KeyError(hk)

            # Queue order: sync [wq k0, xT0 k0-7, xT(tau+1)...];
            # scalar [stair, xT0 k8-15, wk, wo]; gpsimd [wq k1-15, wv,
            # out stores].
            load_w("wq", wq)
            emit_xt_load(0)
            load_w("wk", wk)
            load_w("wv", wv)
            wo_sb = persist.tile([128, HPC, H], bf16, tag="wo")

            qT_sb = persist.tile([128, HPC, S], bf16, tag="qT")
            kT_sb = persist.tile([128, HPC, S], bf16, tag="kT")
            v_sb = persist.tile([128, NK, DPC], bf16, tag="v")
            oT_sb = persist.tile([128, HPC, S], bf16, tag="oT")

            ones_bf_sb = persist.tile([128, 1], bf16, tag="ones_bf")
            nc.vector.memset(ones_bf_sb[:], 1.0)
            onesrow_sb = persist.tile([1, 128], bf16, tag="ones_row")
            nc.vector.memset(onesrow_sb[:], 1.0)

            def emit_qk_proj(tau, wname, h):
                tsl = slice(tau * TBLK, (tau + 1) * TBLK)
                dst = qT_sb if wname == "wq" else kT_sb
                ps = ps_work.tile([128, TBLK], fp32, tag="ps")
                for hk in range(HKT):
                    nc.tensor.matmul(
                        ps[:],
                        lhsT=w_chunk(wname, hk)[:, h * HD:(h + 1) * HD],
                        rhs=xt_chunk(tau, hk),
                        start=(hk == 0), stop=(hk == HKT - 1))
                nc.vector.tensor_copy(out=dst[:, h, tsl], in_=ps[:])

            def emit_v_proj(tau, tb_local):
                ps = ps_work.tile([128, TBLK], fp32, tag="ps")
                for hk in range(HKT):
                    nc.tensor.matmul(
                        ps[:],
                        lhsT=xt_chunk(tau, hk)[:, tb_local * KBLK:(tb_local + 1) * KBLK],
                        rhs=w_chunk("wv", hk),
                        start=(hk == 0), stop=(hk == HKT - 1))
                nc.vector.tensor_copy(
                    out=v_sb[:, tau * (TBLK // KBLK) + tb_local, :], in_=ps[:])

            mask_tiles = {}

            def emit_mask_loads(tau):
                tsl = slice(tau * TBLK, (tau + 1) * TBLK)
                for run in all_runs[tau]:
                    mt = mask_pool.tile([128, max_run_len, TBLK], bf16,
                                        tag="mask")
                    nc.sync.dma_start(
                        mt[:, :len(run), :],
                        maskT.ap()[run[0] * KBLK:(run[-1] + 1) * KBLK, tsl]
                        .rearrange("(k p) t -> p k t", p=128))
                    for j, Tb in enumerate(run):
                        mask_tiles[(tau, Tb)] = mt[:, j, :]

            def emit_attention_head(tau, h, chunk=3):
                tsl = slice(tau * TBLK, (tau + 1) * TBLK)
                blocks = pattern[tau]
                od = ps_acc.tile([128, TBLK], fp32, tag="od")
                tree = []  # (level, tile) stack for streaming bf16 sum tree
                for i, (Tb, mode) in enumerate(blocks):
                    if i and i % chunk == 0:
                        yield
                    delta = 0
                    if mode == MODE_AFFINE:
                        delta = Tb * KBLK - tau * TBLK
                        assert delta in (0, KBLK, 2 * KBLK, 3 * KBLK), delta
                    # the first matmul of the od group must cover the
                    # full width so start=True initializes every column
                    mm_delta = 0 if i == 0 else delta
                    sp = ps_score.tile([128, TBLK], fp32, tag="sc")
                    nc.tensor.matmul(
                        sp[:, mm_delta:],
                        lhsT=kT_sb[:, h, Tb * KBLK:(Tb + 1) * KBLK],
                        rhs=qT_sb[:, h, tau * TBLK + mm_delta:(tau + 1) * TBLK],
                        start=True, stop=True)
                    e = e_pool.tile([128, TBLK], bf16, tag="e")
                    if delta:
                        nc.gpsimd.memset(e[:, :delta], 0.0)
                    nc.scalar.activation(out=e[:, delta:], in_=sp[:, delta:],
                                         func=Exp, scale=inv_sqrt_hd)
                    if mode == MODE_AFFINE:
                        # zero entries above the causal diagonal in the
                        # 128-wide boundary strip: stair[p, j] = (j >= p)
                        nc.vector.tensor_mul(
                            e[:, delta:delta + KBLK],
                            e[:, delta:delta + KBLK], stair_sb[:])
                    elif mode == MODE_LOADMASK:
                        nc.vector.tensor_mul(e[:], e[:], mask_tiles[(tau, Tb)])
                    cur, lvl = e, 0
                    while tree and tree[-1][0] == lvl:
                        _, prev = tree.pop()
                        acc = esum_pool.tile([128, TBLK], bf16, tag="esum")
                        nc.vector.tensor_add(acc[:], prev[:], cur[:])
                        cur, lvl = acc, lvl + 1
                    tree.append((lvl, cur))
                    nc.tensor.matmul(
                        od[:, mm_delta:],
                        lhsT=v_sb[:, Tb, h * HD:(h + 1) * HD],
                        rhs=e[:, mm_delta:],
                        start=(i == 0), stop=(i == len(blocks) - 1),
                        skip_group_check=True)
                while len(tree) > 1:
                    _, a = tree.pop()
                    _, b2 = tree.pop()
                    acc = esum_pool.tile([128, TBLK], bf16, tag="esum")
                    nc.vector.tensor_add(acc[:], a[:], b2[:])
                    tree.append((99, acc))
                esum = tree.pop()[1]
                dn = ps_score.tile([1, TBLK], fp32, tag="sc")
                nc.tensor.matmul(dn[:], lhsT=ones_bf_sb[:], rhs=esum[:],
                                 start=True, stop=True)
                r = r_pool.tile([1, TBLK], fp32, tag="r")
                nc.vector.reciprocal_approx_fast(out=r[:], in_=dn[:])
                rb = rb_pool.tile([1, TBLK], bf16, tag="rb")
                nc.vector.tensor_copy(out=rb[:], in_=r[:])
                # partition-broadcast of the reciprocal row on the PE:
                # out[p, t] = ones[0, p] * rb[0, t].  DVE can read only
                # one PSUM operand per op, so bounce Rp through SBUF.
                Rp = ps_score.tile([128, TBLK], fp32, tag="sc")
                nc.tensor.matmul(Rp[:], lhsT=onesrow_sb[:], rhs=rb[:],
                                 start=True, stop=True)
                R_sb = R_pool.tile([128, TBLK], bf16, tag="R")
                nc.vector.tensor_copy(out=R_sb[:], in_=Rp[:])
                nc.vector.tensor_mul(oT_sb[:, h, tsl], od[:], R_sb[:])

            def emit_out_half_row(tt, half):
                # half of a 128-row slab of the final projection
                for mbl in range(H // TBLK // 2):
                    mb = half * (H // TBLK // 2) + mbl
                    ps = ps_work.tile([128, TBLK], fp32, tag="ps")
                    for h in range(HPC):
                        nc.tensor.matmul(
                            ps[:],
                            lhsT=oT_sb[:, h, tt * 128:(tt + 1) * 128],
                            rhs=wo_sb[:, h, mb * TBLK:(mb + 1) * TBLK],
                            start=(h == 0), stop=(h == HPC - 1))
                    osb = out_pool.tile([128, TBLK], bf16, tag="osb")
                    nc.vector.tensor_copy(out=osb[:], in_=ps[:])
                    nc.gpsimd.dma_start(
                        out.ap()[tt * 128:(tt + 1) * 128,
                                 mb * TBLK:(mb + 1) * TBLK],
                        osb[:])

            def emit_out_row(tt):
                for half in range(2):
                    emit_out_half_row(tt, half)

            # ---- emission schedule -----------------------------------
            # out-projection rows by slot: slot s runs after attention
            # tau=s-1, so a row from ptau may go to any slot > ptau.
            # slot 3 (the longest ACT-bound attention stretch) gets six
            # rows of PE filler; slot 4 (after all attention) gets the
            # four rows that need tau=3's oT.
            p3_assign = {0: [], 1: [0, 1], 2: [2, 3, 4, 5],
                         3: [6, 7, 8, 9, 10, 11], 4: [12, 13, 14, 15]}

            # tau=0 projections sequential q, k, v -- matches the
            # arrival order of wq (gpsimd head), wk (scalar), wv
            # (gpsimd tail) so each phase's weights land just in time.
            for h in range(HPC):
                emit_qk_proj(0, "wq", h)
            for h in range(HPC):
                emit_qk_proj(0, "wk", h)
            for tb in range(HPC):
                emit_v_proj(0, tb)

            emit_mask_loads(0)

            for tau in range(NT):
                fillers = []
                if tau + 1 < NT:
                    emit_xt_load(tau + 1)
                    emit_mask_loads(tau + 1)
                    for h in range(HPC):
                        fillers.append(
                            lambda h=h, t=tau + 1: emit_qk_proj(t, "wq", h))
                        fillers.append(
                            lambda h=h, t=tau + 1: emit_qk_proj(t, "wk", h))
                        fillers.append(
                            lambda h=h, t=tau + 1: emit_v_proj(t, h))
                for tt in p3_assign[tau]:
                    fillers += [lambda tt=tt, hf=hf: emit_out_half_row(tt, hf)
                                for hf in range(2)]
                # hold a reserve back so the PE has independent work
                # while the final head's normalization chain completes
                n_reserve = min(2, len(fillers))
                paced = fillers[:len(fillers) - n_reserve]
                reserve = fillers[len(fillers) - n_reserve:]
                n_yields = HPC * max(1, (len(pattern[tau]) + 2) // 3)
                done = 0
                yi = 0
                for h in range(HPC):
                    for _ in emit_attention_head(tau, h):
                        yi += 1
                        target = min(len(paced), yi * len(paced) // max(1, n_yields))
                        while done < target:
                            paced[done]()
                            done += 1
                    yi += 1
                while done < len(paced):
                    paced[done]()
                    done += 1
                for f in reserve:
                    f()
                if tau == 0:
                    # wo rides the scalar queue behind tau=0's exps --
                    # landed long before the first slot-1 out row.
                    nc.scalar.dma_start(
                        wo_sb[:],
                        wo.ap().rearrange("(c p) m -> p c m", p=128))

            for tt in p3_assign[NT]:
                emit_out_row(tt)

    nc.compile()
    return nc


def _classify(mask):
    """Per 128x512 block of mask^T: skip / full / affine / partial,
    unioned over batches.  Returns the pattern tuple, or None if some
    row is fully masked (degenerate -- reference gives uniform weights
    there)."""
    if not mask.any(axis=2).all():
        return None
    tr = np.arange(TBLK)[:, None]
    Tr = np.arange(KBLK)[None, :]
    pattern = []
    for tau in range(NT):
        blocks = []
        for Tb in range(NK):
            blk = mask[:, tau * TBLK:(tau + 1) * TBLK,
                       Tb * KBLK:(Tb + 1) * KBLK]
            if not blk.any():
                continue
            if blk.all():
                blocks.append((Tb, MODE_FULL))
                continue
            # causal staircase? mask[t, T] = (t >= T)
            stair = (tau * TBLK + tr) >= (Tb * KBLK + Tr)
            if all((blk[b] == stair).all() for b in range(blk.shape[0])) \
                    and 0 <= Tb * KBLK - tau * TBLK < TBLK:
                blocks.append((Tb, MODE_AFFINE))
            else:
                blocks.append((Tb, MODE_LOADMASK))
        pattern.append(tuple(blocks))
    return tuple(pattern)


def _reference_fallback(x, mask, Wq, Wk, Wv, Wo):
    out = np.empty((B, S, H), np.float32)
    for b in range(B):
        q = (x[b] @ Wq).reshape(S, NH, HD).transpose(1, 0, 2)
        k = (x[b] @ Wk).reshape(S, NH, HD).transpose(1, 0, 2)
        v = (x[b] @ Wv).reshape(S, NH, HD).transpose(1, 0, 2)
        s = np.einsum("htd,hTd->htT", q, k) / np.sqrt(HD)
        s = np.where(mask[b][None], s, -1e10)
        s -= s.max(-1, keepdims=True)
        w = np.exp(s)
        w /= w.sum(-1, keepdims=True)
        o = np.einsum("htT,hTd->htd", w, v)
        out[b] = o.transpose(1, 0, 2).reshape(S, NH * HD) @ Wo
    return out


def kernel(x, mask, Wq, Wk, Wv, Wo):
    x = np.asarray(x, np.float32)
    mask = np.asarray(mask).astype(bool)
    Wq = np.asarray(Wq, np.float32)
    Wk = np.asarray(Wk, np.float32)
    Wv = np.asarray(Wv, np.float32)
    Wo = np.asarray(Wo, np.float32)
    assert x.shape == (B, S, H) and mask.shape == (B, S, S)

    pattern = _classify(mask)
    if pattern is None:
        return _reference_fallback(x, mask, Wq, Wk, Wv, Wo)

    if pattern not in _kernel_cache:
        _kernel_cache[pattern] = _build(pattern)
    nc = _kernel_cache[pattern]

    xT_b = [np.ascontiguousarray(x[b].T).astype(_BF16) for b in range(B)]
    maskT_b = [np.ascontiguousarray(mask[b].T).astype(_BF16) for b in range(B)]
    wq_g = [np.ascontiguousarray(Wq[:, g * DPC:(g + 1) * DPC]).astype(_BF16)
            for g in range(GROUPS)]
    wk_g = [np.ascontiguousarray(Wk[:, g * DPC:(g + 1) * DPC]).astype(_BF16)
            for g in range(GROUPS)]
    wv_g = [np.ascontiguousarray(Wv[:, g * DPC:(g + 1) * DPC]).astype(_BF16)
            for g in range(GROUPS)]
    wo_g = [np.ascontiguousarray(Wo[g * DPC:(g + 1) * DPC, :]).astype(_BF16)
            for g in range(GROUPS)]
    stair_np = (np.arange(KBLK)[None, :] >= np.arange(KBLK)[:, None]) \
        .astype(_BF16)

    in_maps = []
    for i in range(N_CORES):
        b, g = divmod(i, GROUPS)
        in_maps.append({
            "xT": xT_b[b], "maskT": maskT_b[b], "stair": stair_np,
            "wq": wq_g[g], "wk": wk_g[g], "wv": wv_g[g], "wo": wo_g[g],
        })

    from concourse.bass_utils import run_bass_kernel_spmd
    res = run_bass_kernel_spmd(nc, in_maps, core_ids=list(range(N_CORES)))

    out = np.zeros((B, S, H), np.float32)
    for i in range(N_CORES):
        b = i // GROUPS
        out[b] += res.results[i]["out"].astype(np.float32)
    return out
